# revision 8
# baseline (speedup 1.0000x reference)
"""LocalAttention (B=1, S=4096, D=1024, H=16, hd=64, window=128) on 8 trn2 cores.

Sharding: sequence-parallel. Core c owns queries [512c, 512c+512) and receives
a key/value halo slice of 768 rows ([512c-128, 512c+640), zero-padded at the
global edges). All projection weights are replicated (bf16). Everything on
device runs in bf16 with fp32 PSUM accumulation.

Per-core dataflow (v4):
  Fine-grained per-slab input DMAs ordered exactly as the Q projection
  consumes them. qT = (Wq^T x^T) in [e, s] layout; kT likewise over the
  768-col halo range; v = (Vin Wv) in natural [s, e] layout + a ones-column
  per head (softmax denominator rides along col 64).
  Scores are computed kb-major: for each (head, key-block) ONE matmul
  [64 x (128k, W q)] where W spans the 1..3 query blocks within the window.
  Exp (scale=1/8, bf16, no max-subtract), then bf16 multiplicative masks on
  only the 1-2 diagonal-adjacent 128-col blocks of each tile (per-core host
  data bakes in the window triangles and the global-edge zero blocks).
  All V-projection chains run interleaved with the kb0/kb1 score passes so
  the PE stream stays dense. The rolling qb loop then does: new score tile
  (kb=qb+2) per head, lag-1 PV (3 accumulating matmuls -> [128,65], col 64 =
  denominator, DVE reciprocal + tensor_scalar normalize into ao[qb]), and as
  each head PAIR finishes, its [q,128] slice of ao is XBAR-transposed (DMA
  engine, no PE) into aot and the output-projection PSUM chains (one per
  512-col half) advance by one e-block - so the out proj finishes almost
  together with the last head instead of serially after it.
"""

import os

import numpy as np
import ml_dtypes

import concourse.bass as bass
import concourse.bacc as bacc
import concourse.mybir as mybir
import concourse.tile as tile
from concourse.bass_utils import run_bass_kernel_spmd

BF16 = mybir.dt.bfloat16
FP32 = mybir.dt.float32

NCORES = 8
S = 4096
D = 1024
H = 16
HD = 64
E = H * HD  # 1024
WIN = 128
SL = S // NCORES       # 512 queries per core
SK = SL + 2 * WIN      # 768 keys/values incl. halo
NQB = SL // 128        # 4 query blocks
NKB = SK // 128        # 6 key blocks
NDB = D // 128         # 8 contraction blocks
NEB = E // 128         # 8 embed blocks
VROW = HD + 1          # 65: v columns per head incl. ones column

# kb-major score tiles: valid q-blocks for key-block kb are
# [max(0, kb-2), min(NQB-1, kb)] (window = +-1 block around diagonal).
KB_Q0 = [max(0, kb - 2) for kb in range(NKB)]
KB_QN = [min(NQB - 1, kb) - max(0, kb - 2) + 1 for kb in range(NKB)]

_CACHE = {}
LAST_RESULT = None  # BassKernelResults of the most recent run (for test.py)


def _build_nc():
    nc = bacc.Bacc("TRN2", target_bir_lowering=False, debug=False)

    qt_d = nc.dram_tensor("qt", [D, SL], BF16, kind="ExternalInput").ap()
    kt_d = nc.dram_tensor("kt", [D, SK], BF16, kind="ExternalInput").ap()
    vt_d = nc.dram_tensor("vt", [D, SK], BF16, kind="ExternalInput").ap()
    wq_d = nc.dram_tensor("wq", [D, E], BF16, kind="ExternalInput").ap()
    wk_d = nc.dram_tensor("wk", [D, E], BF16, kind="ExternalInput").ap()
    wv_d = nc.dram_tensor("wv", [D, E], BF16, kind="ExternalInput").ap()
    wo_d = nc.dram_tensor("wo", [E, D], BF16, kind="ExternalInput").ap()
    # bf16 multiplicative masks: [0:128]=m2, [128:256]=m0, [256:384]=kb0
    # block (m0 or zeros at core 0), [384:512]=kb5 block (m2 or zeros at
    # core 7).
    msk_d = nc.dram_tensor("msk", [128, 512], BF16, kind="ExternalInput").ap()
    out_d = nc.dram_tensor("out", [SL, D], FP32, kind="ExternalOutput").ap()

    with tile.TileContext(nc) as tc:
        pools = []

        def pool(name, bufs, **kw):
            p = tc.tile_pool(name=name, bufs=bufs, **kw)
            pools.append(p)
            return p.__enter__()

        const = pool("const", 1)
        psum = pool("psum", 2, space="PSUM")       # proj + out-proj chains
        pscore = pool("pscore", 4, space="PSUM")   # score tiles [128, <=384]
        ppv_pool = pool("ppv", 2, space="PSUM")    # PV tiles [128, 65]
        ep = pool("expp", 3)                       # per-(h,kb) exp tiles
        aop = pool("ao", 2)                        # per-qb attn-out [q, e]
        aotp = pool("aot", 2)                      # per-qb transposed [e, q]
        op = pool("o", 2)                          # per-qb fp32 out staging
        rp = pool("recip", 8)

        # ---- persistent SBUF tensors ----
        wq_sb = const.tile([128, NDB * E], BF16, tag="wq")
        wk_sb = const.tile([128, NDB * E], BF16, tag="wk")
        wv_sb = const.tile([128, NDB * E], BF16, tag="wv")
        wo_sb = const.tile([128, NEB * D], BF16, tag="wo")
        qtin_sb = const.tile([128, NDB * SL], BF16, tag="qtin")
        ktin_sb = const.tile([128, NDB * SK], BF16, tag="ktin")
        vtin_sb = const.tile([128, NDB * SK], BF16, tag="vtin")
        qt_sb = const.tile([128, NEB * SL], BF16, tag="qt")    # [e,s] per e-blk
        kt_sb = const.tile([128, NEB * SK], BF16, tag="kt")
        v_sb = const.tile([128, NKB * H * VROW], BF16, tag="v")  # [s, h*65]
        msk_sb = const.tile([128, 512], BF16, tag="msk")

        sync = nc.sync

        # ---- input DMAs: per-slab, ordered as consumed ----
        def load_slab(sb, dr, ncols, b):
            sync.dma_start(
                sb[:, b * ncols:(b + 1) * ncols],
                dr[b * 128:(b + 1) * 128],
            )

        for db in range(NDB):
            load_slab(qtin_sb, qt_d, SL, db)
            load_slab(wq_sb, wq_d, E, db)
        for db in range(NDB):
            load_slab(ktin_sb, kt_d, SK, db)
            load_slab(wk_sb, wk_d, E, db)
        sync.dma_start(msk_sb[:], msk_d[:])
        for db in range(NDB):
            load_slab(vtin_sb, vt_d, SK, db)
            load_slab(wv_sb, wv_d, E, db)
        for eb in range(NEB):
            load_slab(wo_sb, wo_d, D, eb)

        # ones columns of v_sb (col hd=64 of each head group)
        v3 = v_sb[:].rearrange("p (k h c) -> p k h c", k=NKB, h=H)
        nc.gpsimd.memset(v3[:, :, :, HD:VROW], 1.0)

        # ---- q projection: [e, s] = Wq[d,e].T @ QT[d,s] ----
        for eb in range(NEB):
            ps = psum.tile([128, 512], FP32, tag="ps")
            for db in range(NDB):
                nc.tensor.matmul(
                    ps[:],
                    lhsT=wq_sb[:, db * E + eb * 128: db * E + (eb + 1) * 128],
                    rhs=qtin_sb[:, db * SL: db * SL + SL],
                    start=(db == 0),
                    stop=(db == NDB - 1),
                )
            nc.vector.tensor_copy(qt_sb[:, eb * SL:(eb + 1) * SL], ps[:])

        # ---- k projection: [e, s] = Wk[d,e].T @ KT[d,s] over halo range ----
        for eb in range(NEB):
            for s0, s1 in ((0, 512), (512, SK)):
                ps = psum.tile([128, 512], FP32, tag="ps")
                for db in range(NDB):
                    nc.tensor.matmul(
                        ps[:, : s1 - s0],
                        lhsT=wk_sb[:, db * E + eb * 128: db * E + (eb + 1) * 128],
                        rhs=ktin_sb[:, db * SK + s0: db * SK + s1],
                        start=(db == 0),
                        stop=(db == NDB - 1),
                    )
                nc.vector.tensor_copy(
                    kt_sb[:, eb * SK + s0: eb * SK + s1], ps[:, : s1 - s0]
                )

        # ---- v projection chain: one (kb, eh) psum chain ----
        def v_chain(kb, eh):
            ps = psum.tile([128, 512], FP32, tag="ps")
            for db in range(NDB):
                nc.tensor.matmul(
                    ps[:],
                    lhsT=vtin_sb[:, db * SK + kb * 128: db * SK + (kb + 1) * 128],
                    rhs=wv_sb[:, db * E + eh * 512: db * E + (eh + 1) * 512],
                    start=(db == 0),
                    stop=(db == NDB - 1),
                )
            dst = v3[:, kb, eh * 8:(eh + 1) * 8, 0:HD]
            src = ps[:].rearrange("p (h c) -> p h c", c=HD)
            nc.scalar.copy(dst, src)

        # ---- attention pieces ----
        scale = 1.0 / np.sqrt(HD)
        expp_tiles = {}  # (h, kb) -> sbuf tile [128, W]

        def score_tile(h, kb):
            """One score matmul + exp + block masks for (head, key block)."""
            hp = (h % 2) * HD
            he = h // 2
            w = KB_QN[kb] * 128
            q0 = KB_Q0[kb] * 128
            pscr = pscore.tile([128, 384], FP32, tag="scr")
            nc.tensor.matmul(
                pscr[:, :w],
                lhsT=kt_sb[hp:hp + HD, he * SK + kb * 128: he * SK + (kb + 1) * 128],
                rhs=qt_sb[hp:hp + HD, he * SL + q0: he * SL + q0 + w],
                start=True,
                stop=True,
            )
            expp = ep.tile([128, 384], BF16, tag=f"expp{h}")
            nc.scalar.activation(
                expp[:, :w], pscr[:, :w],
                mybir.ActivationFunctionType.Exp, scale=scale,
            )
            if kb == 0:      # qb0 block (r=0): m0, or zeros on core 0
                nc.vector.tensor_mul(
                    expp[:, 0:128], expp[:, 0:128], msk_sb[:, 256:384]
                )
            elif kb == 1:    # qb1 block (r=0) at cols 128:256: m0
                nc.vector.tensor_mul(
                    expp[:, 128:256], expp[:, 128:256], msk_sb[:, 128:256]
                )
            elif kb in (2, 3):  # [m2 | . | m0] via one strided op
                ev = expp[:, 0:384].rearrange("p (b c) -> p b c", b=3)
                dst = ev[:, 0::2]
                src = msk_sb[:, 0:256].rearrange("p (b c) -> p b c", b=2)
                nc.vector.tensor_mul(dst, dst, src)
            elif kb == 4:    # qb2 block (r=2): m2
                nc.vector.tensor_mul(
                    expp[:, 0:128], expp[:, 0:128], msk_sb[:, 0:128]
                )
            else:            # kb == 5: qb3 block (r=2): m2, or zeros core 7
                nc.vector.tensor_mul(
                    expp[:, 0:128], expp[:, 0:128], msk_sb[:, 384:512]
                )
            expp_tiles[(h, kb)] = expp

        def pv_norm(h, qb, ao):
            """PV + normalize for one (head, q-block) into ao tile."""
            ppv = ppv_pool.tile([128, VROW], FP32, tag="pv")
            for r in range(3):
                kb = qb + r
                idx = qb - KB_Q0[kb]
                nc.tensor.matmul(
                    ppv[:],
                    lhsT=expp_tiles[(h, kb)][:, idx * 128:(idx + 1) * 128],
                    rhs=v_sb[:, (kb * H + h) * VROW:(kb * H + h + 1) * VROW],
                    start=(r == 0),
                    stop=(r == 2),
                )
            rd = rp.tile([128, 1], FP32, tag="rd")
            nc.vector.reciprocal(rd[:], ppv[:, HD:VROW])
            nc.vector.tensor_scalar(
                ao[:, h * HD:(h + 1) * HD],
                ppv[:, 0:HD],
                rd[:],
                None,
                op0=mybir.AluOpType.mult,
            )

        # ---- prologue attention: kb0/kb1 score tiles + ALL V chains ----
        vjobs = [(kb, eh) for kb in range(NKB) for eh in range(2)]
        vi = 0
        for pas in range(2):           # kb0 pass, kb1 pass
            for h in range(H):
                score_tile(h, pas)
                if h % 3 == 2 and vi < len(vjobs):
                    v_chain(*vjobs[vi]); vi += 1
        while vi < len(vjobs):
            v_chain(*vjobs[vi]); vi += 1

        # ---- rolling qb loop; out proj of qb-1 interleaved into qb ----
        def make_outproj(qbp, aotp_tile):
            psA = psum.tile([128, 512], FP32, tag="ps")
            psB = psum.tile([128, 512], FP32, tag="ps")

            def mm(eb):
                for dh, ps in ((0, psA), (1, psB)):
                    nc.tensor.matmul(
                        ps[:],
                        lhsT=aotp_tile[:, eb * 128:(eb + 1) * 128],
                        rhs=wo_sb[:, eb * D + dh * 512: eb * D + (dh + 1) * 512],
                        start=(eb == 0),
                        stop=(eb == NEB - 1),
                    )

            def finish():
                o_t = op.tile([128, D], FP32, tag="o")
                for dh, ps in ((0, psA), (1, psB)):
                    dst = o_t[:, dh * 512:(dh + 1) * 512]
                    if dh == 0:
                        nc.vector.tensor_copy(dst, ps[:])
                    else:
                        nc.scalar.copy(dst, ps[:])
                    sync.dma_start(
                        out_d[qbp * 128:(qbp + 1) * 128,
                              dh * 512:(dh + 1) * 512],
                        dst,
                    )

            return mm, finish

        prev_proj = None  # (mm, finish) for qb-1
        for qb in range(NQB):
            ao = aop.tile([128, E], BF16, tag="ao")
            kbn = qb + 2
            for h in range(H):
                score_tile(h, kbn)
                if h >= 1:
                    pv_norm(h - 1, qb, ao)
                if prev_proj is not None and h % 2 == 1:
                    prev_proj[0]((h - 1) // 2)       # e-blocks 0..7
            pv_norm(H - 1, qb, ao)
            if prev_proj is not None:
                prev_proj[1]()
            aot = aotp.tile([128, E], BF16, tag="aot")
            sync.dma_start_transpose(
                aot[:].rearrange("p (b q) -> p b q", q=128), ao[:]
            )
            prev_proj = make_outproj(qb, aot)

        # final out proj (qb3)
        for eb in range(NEB):
            prev_proj[0](eb)
        prev_proj[1]()

        for p in reversed(pools):
            p.__exit__(None, None, None)

    nc.compile()
    return nc


def _host_masks():
    bf = ml_dtypes.bfloat16
    kt = np.arange(128)[:, None]
    qi = np.arange(128)[None, :]
    tri0 = (qi <= kt).astype(bf)          # r=0 keep
    tri2 = (kt <= qi).astype(bf)          # r=2 keep
    zeros = np.zeros((128, 128), bf)

    masks = []
    for c in range(NCORES):
        m = np.empty((128, 512), bf)
        m[:, 0:128] = tri2
        m[:, 128:256] = tri0
        m[:, 256:384] = zeros if c == 0 else tri0
        m[:, 384:512] = zeros if c == NCORES - 1 else tri2
        masks.append(m)
    return masks


def _host_inputs(query, key, value, Wq, Wk, Wv, Wo):
    bf = ml_dtypes.bfloat16
    q2 = np.ascontiguousarray(query.reshape(S, D))
    k2 = np.asarray(key).reshape(S, D)
    v2 = np.asarray(value).reshape(S, D)
    kpad = np.zeros((S + 2 * WIN, D), np.float32)
    kpad[WIN:WIN + S] = k2
    vpad = np.zeros((S + 2 * WIN, D), np.float32)
    vpad[WIN:WIN + S] = v2

    wq = np.ascontiguousarray(Wq.astype(bf))
    wk = np.ascontiguousarray(Wk.astype(bf))
    wv = np.ascontiguousarray(Wv.astype(bf))
    wo = np.ascontiguousarray(Wo.astype(bf))
    masks = _host_masks()

    in_maps = []
    for c in range(NCORES):
        s0 = c * SL
        qt = np.ascontiguousarray(q2[s0:s0 + SL].T.astype(bf))
        ktc = np.ascontiguousarray(kpad[s0:s0 + SK].T.astype(bf))
        vtc = np.ascontiguousarray(vpad[s0:s0 + SK].T.astype(bf))
        in_maps.append({
            "qt": qt, "kt": ktc, "vt": vtc,
            "wq": wq, "wk": wk, "wv": wv, "wo": wo,
            "msk": masks[c],
        })
    return in_maps


def kernel(query, key, value, Wq, Wk, Wv, Wo):
    global LAST_RESULT
    if "nc" not in _CACHE:
        _CACHE["nc"] = _build_nc()
    nc = _CACHE["nc"]
    in_maps = _host_inputs(
        np.asarray(query), np.asarray(key), np.asarray(value),
        np.asarray(Wq), np.asarray(Wk), np.asarray(Wv), np.asarray(Wo),
    )
    trace = os.environ.get("KERNEL_TRACE", "0") == "1"
    try:
        res = run_bass_kernel_spmd(
            nc, in_maps, core_ids=list(range(NCORES)), trace=trace
        )
    except ModuleNotFoundError:
        res = run_bass_kernel_spmd(
            nc, in_maps, core_ids=list(range(NCORES)), trace=False
        )
    LAST_RESULT = res
    out = np.concatenate([res.results[c]["out"] for c in range(NCORES)], axis=0)
    return out.reshape(1, S, D).astype(np.float32)


# revision 16
# speedup vs baseline: 1.0274x; 1.0274x over previous
"""LocalAttention (B=1, S=4096, D=1024, H=16, hd=64, window=128) on 8 trn2 cores.

Sharding: sequence-parallel. Core c owns queries [512c, 512c+512) and receives
a key/value halo slice of 768 rows ([512c-128, 512c+640), zero-padded at the
global edges). All projection weights are replicated (bf16). Everything on
device runs in bf16 with fp32 PSUM accumulation.

Per-core dataflow (v4):
  Fine-grained per-slab input DMAs ordered exactly as the Q projection
  consumes them. qT = (Wq^T x^T) in [e, s] layout; kT likewise over the
  768-col halo range; v = (Vin Wv) in natural [s, e] layout + a ones-column
  per head (softmax denominator rides along col 64).
  Scores are computed kb-major: for each (head, key-block) ONE matmul
  [64 x (128k, W q)] where W spans the 1..3 query blocks within the window.
  Exp (scale=1/8, bf16, no max-subtract), then bf16 multiplicative masks on
  only the 1-2 diagonal-adjacent 128-col blocks of each tile (per-core host
  data bakes in the window triangles and the global-edge zero blocks).
  All V-projection chains run interleaved with the kb0/kb1 score passes so
  the PE stream stays dense. The rolling qb loop then does: new score tile
  (kb=qb+2) per head, lag-1 PV (3 accumulating matmuls -> [128,65], col 64 =
  denominator, DVE reciprocal + tensor_scalar normalize into ao[qb]), and as
  each head PAIR finishes, its [q,128] slice of ao is XBAR-transposed (DMA
  engine, no PE) into aot and the output-projection PSUM chains (one per
  512-col half) advance by one e-block - so the out proj finishes almost
  together with the last head instead of serially after it.
"""

import os

import numpy as np
import ml_dtypes

import concourse.bass as bass
import concourse.bacc as bacc
import concourse.mybir as mybir
import concourse.tile as tile
from concourse.bass_utils import run_bass_kernel_spmd

BF16 = mybir.dt.bfloat16
FP32 = mybir.dt.float32

NCORES = 8
S = 4096
D = 1024
H = 16
HD = 64
E = H * HD  # 1024
WIN = 128
SL = S // NCORES       # 512 queries per core
SK = SL + 2 * WIN      # 768 keys/values incl. halo
NQB = SL // 128        # 4 query blocks
NKB = SK // 128        # 6 key blocks
NDB = D // 128         # 8 contraction blocks
NEB = E // 128         # 8 embed blocks
VROW = HD + 1          # 65: v columns per head incl. ones column

# kb-major score tiles: valid q-blocks for key-block kb are
# [max(0, kb-2), min(NQB-1, kb)] (window = +-1 block around diagonal).
KB_Q0 = [max(0, kb - 2) for kb in range(NKB)]
KB_QN = [min(NQB - 1, kb) - max(0, kb - 2) + 1 for kb in range(NKB)]
KB_OFF = np.cumsum([0] + [n * 128 for n in KB_QN]).tolist()  # col offsets
SCORE_COLS = KB_OFF[-1]  # 1536

_CACHE = {}
LAST_RESULT = None  # BassKernelResults of the most recent run (for test.py)


def _build_nc():
    nc = bacc.Bacc("TRN2", target_bir_lowering=False, debug=False)

    qt_d = nc.dram_tensor("qt", [D, SL], BF16, kind="ExternalInput").ap()
    kt_d = nc.dram_tensor("kt", [D, SK], BF16, kind="ExternalInput").ap()
    vt_d = nc.dram_tensor("vt", [D, SK], BF16, kind="ExternalInput").ap()
    wq_d = nc.dram_tensor("wq", [D, E], BF16, kind="ExternalInput").ap()
    wk_d = nc.dram_tensor("wk", [D, E], BF16, kind="ExternalInput").ap()
    wv_d = nc.dram_tensor("wv", [D, E], BF16, kind="ExternalInput").ap()
    wo_d = nc.dram_tensor("wo", [E, D], BF16, kind="ExternalInput").ap()
    # bf16 multiplicative masks: [0:128]=m2, [128:256]=m0, [256:384]=kb0
    # block (m0 or zeros at core 0), [384:512]=kb5 block (m2 or zeros at
    # core 7).
    msk_d = nc.dram_tensor("msk", [128, 512], BF16, kind="ExternalInput").ap()
    out_d = nc.dram_tensor("out", [SL, D], FP32, kind="ExternalOutput").ap()

    with tile.TileContext(nc) as tc:
        pools = []

        def pool(name, bufs, **kw):
            p = tc.tile_pool(name=name, bufs=bufs, **kw)
            pools.append(p)
            return p.__enter__()

        const = pool("const", 1)
        psum = pool("psum", 2, space="PSUM")       # proj + out-proj chains
        pscore = pool("pscore", 3, space="PSUM")   # score tiles [128, <=384]
        ppv_pool = pool("ppv", 3, space="PSUM")    # PV tiles [128, 65]
        aop = pool("ao", 2)                        # per-qb attn-out [q, e]
        aotp = pool("aot", 2)                      # per-qb transposed [e, q]
        op = pool("o", 2)                          # per-qb fp32 out staging
        rp = pool("recip", 8)

        # ---- persistent SBUF tensors ----
        wq_sb = const.tile([128, NDB * E], BF16, tag="wq")
        wk_sb = const.tile([128, NDB * E], BF16, tag="wk")
        wv_sb = const.tile([128, NDB * E], BF16, tag="wv")
        wo_sb = const.tile([128, NEB * D], BF16, tag="wo")
        qtin_sb = const.tile([128, NDB * SL], BF16, tag="qtin")
        ktin_sb = const.tile([128, NDB * SK], BF16, tag="ktin")
        vtin_sb = const.tile([128, NDB * SK], BF16, tag="vtin")
        qt_sb = const.tile([128, NEB * SL], BF16, tag="qt")    # [e,s] per e-blk
        kt_sb = const.tile([128, NEB * SK], BF16, tag="kt")
        v_sb = const.tile([128, NKB * H * VROW], BF16, tag="v")  # [s, h*65]
        msk_sb = const.tile([128, 512], BF16, tag="msk")
        # fully-resident exp'd score tiles: per head, kb-major packed [128,1536]
        expp_sb = []
        for h in range(H):
            expp_h = const.tile([128, SCORE_COLS], BF16, tag=f"expp{h}",
                                name=f"expp{h}")
            expp_sb.append(expp_h)

        sync = nc.sync

        # ---- input DMAs: per-slab, ordered as consumed ----
        def load_slab(sb, dr, ncols, b):
            sync.dma_start(
                sb[:, b * ncols:(b + 1) * ncols],
                dr[b * 128:(b + 1) * 128],
            )

        for db in range(NDB):
            load_slab(qtin_sb, qt_d, SL, db)
            load_slab(wq_sb, wq_d, E, db)
        for db in range(NDB):
            load_slab(ktin_sb, kt_d, SK, db)
            load_slab(wk_sb, wk_d, E, db)
        sync.dma_start(msk_sb[:], msk_d[:])
        for db in range(NDB):
            load_slab(vtin_sb, vt_d, SK, db)
            load_slab(wv_sb, wv_d, E, db)
        for eb in range(NEB):
            load_slab(wo_sb, wo_d, D, eb)

        # ones columns of v_sb (col hd=64 of each head group)
        v3 = v_sb[:].rearrange("p (k h c) -> p k h c", k=NKB, h=H)
        nc.gpsimd.memset(v3[:, :, :, HD:VROW], 1.0)

        # ---- q projection: [e, s] = Wq[d,e].T @ QT[d,s] ----
        for eb in range(NEB):
            ps = psum.tile([128, 512], FP32, tag="ps")
            for db in range(NDB):
                nc.tensor.matmul(
                    ps[:],
                    lhsT=wq_sb[:, db * E + eb * 128: db * E + (eb + 1) * 128],
                    rhs=qtin_sb[:, db * SL: db * SL + SL],
                    start=(db == 0),
                    stop=(db == NDB - 1),
                )
            nc.vector.tensor_copy(qt_sb[:, eb * SL:(eb + 1) * SL], ps[:])

        # ---- k projection chain: one (eb, chunk) psum chain ----
        def k_chain(eb, s0, s1):
            ps = psum.tile([128, 512], FP32, tag="ps")
            for db in range(NDB):
                nc.tensor.matmul(
                    ps[:, : s1 - s0],
                    lhsT=wk_sb[:, db * E + eb * 128: db * E + (eb + 1) * 128],
                    rhs=ktin_sb[:, db * SK + s0: db * SK + s1],
                    start=(db == 0),
                    stop=(db == NDB - 1),
                )
            nc.vector.tensor_copy(
                kt_sb[:, eb * SK + s0: eb * SK + s1], ps[:, : s1 - s0]
            )

        # ---- v projection chain: one (kb, eh) psum chain ----
        def v_chain(kb, eh):
            ps = psum.tile([128, 512], FP32, tag="ps")
            for db in range(NDB):
                nc.tensor.matmul(
                    ps[:],
                    lhsT=vtin_sb[:, db * SK + kb * 128: db * SK + (kb + 1) * 128],
                    rhs=wv_sb[:, db * E + eh * 512: db * E + (eh + 1) * 512],
                    start=(db == 0),
                    stop=(db == NDB - 1),
                )
            dst = v3[:, kb, eh * 8:(eh + 1) * 8, 0:HD]
            src = ps[:].rearrange("p (h c) -> p h c", c=HD)
            nc.scalar.copy(dst, src)

        # ---- attention pieces ----
        scale = 1.0 / np.sqrt(HD)

        def score_tile(h, kb):
            """One score matmul + exp + block masks for (head, key block)."""
            hp = (h % 2) * HD
            he = h // 2
            w = KB_QN[kb] * 128
            q0 = KB_Q0[kb] * 128
            pscr = pscore.tile([128, 384], FP32, tag="scr")
            nc.tensor.matmul(
                pscr[:, :w],
                lhsT=kt_sb[hp:hp + HD, he * SK + kb * 128: he * SK + (kb + 1) * 128],
                rhs=qt_sb[hp:hp + HD, he * SL + q0: he * SL + q0 + w],
                start=True,
                stop=True,
            )
            expp = expp_sb[h][:, KB_OFF[kb]:KB_OFF[kb] + w]
            nc.scalar.activation(
                expp, pscr[:, :w],
                mybir.ActivationFunctionType.Exp, scale=scale,
            )
            if kb == 0:      # qb0 block (r=0): m0, or zeros on core 0
                nc.vector.tensor_mul(
                    expp[:, 0:128], expp[:, 0:128], msk_sb[:, 256:384]
                )
            elif kb == 1:    # qb1 block (r=0) at cols 128:256: m0
                nc.vector.tensor_mul(
                    expp[:, 128:256], expp[:, 128:256], msk_sb[:, 128:256]
                )
            elif kb in (2, 3):  # [m2 | . | m0] via one strided op
                ev = expp.rearrange("p (b c) -> p b c", b=3)
                dst = ev[:, 0::2]
                src = msk_sb[:, 0:256].rearrange("p (b c) -> p b c", b=2)
                nc.vector.tensor_mul(dst, dst, src)
            elif kb == 4:    # qb2 block (r=2): m2
                nc.vector.tensor_mul(
                    expp[:, 0:128], expp[:, 0:128], msk_sb[:, 0:128]
                )
            else:            # kb == 5: qb3 block (r=2): m2, or zeros core 7
                nc.vector.tensor_mul(
                    expp[:, 0:128], expp[:, 0:128], msk_sb[:, 384:512]
                )

        def pv_norm(h, qb, ao):
            """PV + normalize for one (head, q-block) into ao tile."""
            ppv = ppv_pool.tile([128, VROW], FP32, tag="pv")
            for r in range(3):
                kb = qb + r
                idx = qb - KB_Q0[kb]
                off = KB_OFF[kb] + idx * 128
                nc.tensor.matmul(
                    ppv[:],
                    lhsT=expp_sb[h][:, off:off + 128],
                    rhs=v_sb[:, (kb * H + h) * VROW:(kb * H + h + 1) * VROW],
                    start=(r == 0),
                    stop=(r == 2),
                )
            rd = rp.tile([128, 1], FP32, tag="rd")
            nc.vector.reciprocal(rd[:], ppv[:, HD:VROW])
            nc.vector.tensor_scalar(
                ao[:, h * HD:(h + 1) * HD],
                ppv[:, 0:HD],
                rd[:],
                None,
                op0=mybir.AluOpType.mult,
            )

        # ---- K proj + all score tiles + V proj, phase-interleaved ----
        vjobs = [(kb, eh) for kb in range(NKB) for eh in range(2)]
        vi = 0
        for eb in range(NEB):
            k_chain(eb, 0, 512)
            k_chain(eb, 512, SK)
            for h in (2 * eb, 2 * eb + 1):
                for kb in range(NKB):
                    score_tile(h, kb)
            if eb >= 2:
                v_chain(*vjobs[vi]); vi += 1
        while vi < len(vjobs):
            v_chain(*vjobs[vi]); vi += 1

        # ---- rolling qb loop; out proj of qb-1 interleaved into qb ----
        def make_outproj(qbp, aotp_tile):
            psA = psum.tile([128, 512], FP32, tag="ps")
            psB = psum.tile([128, 512], FP32, tag="ps")

            def mm(eb):
                for dh, ps in ((0, psA), (1, psB)):
                    nc.tensor.matmul(
                        ps[:],
                        lhsT=aotp_tile[:, eb * 128:(eb + 1) * 128],
                        rhs=wo_sb[:, eb * D + dh * 512: eb * D + (dh + 1) * 512],
                        start=(eb == 0),
                        stop=(eb == NEB - 1),
                    )

            def finish():
                o_t = op.tile([128, D], FP32, tag="o")
                for dh, ps in ((0, psA), (1, psB)):
                    dst = o_t[:, dh * 512:(dh + 1) * 512]
                    if dh == 0:
                        nc.vector.tensor_copy(dst, ps[:])
                    else:
                        nc.scalar.copy(dst, ps[:])
                    sync.dma_start(
                        out_d[qbp * 128:(qbp + 1) * 128,
                              dh * 512:(dh + 1) * 512],
                        dst,
                    )

            return mm, finish

        prev_proj = None  # (mm, finish) for qb-1
        for qb in range(NQB):
            ao = aop.tile([128, E], BF16, tag="ao")
            for h in range(H):
                pv_norm(h, qb, ao)
                if prev_proj is not None and h % 2 == 1:
                    prev_proj[0]((h - 1) // 2)       # e-blocks 0..7
            if prev_proj is not None:
                prev_proj[1]()
            aot = aotp.tile([128, E], BF16, tag="aot")
            sync.dma_start_transpose(
                aot[:].rearrange("p (b q) -> p b q", q=128), ao[:]
            )
            prev_proj = make_outproj(qb, aot)

        # final out proj (qb3)
        for eb in range(NEB):
            prev_proj[0](eb)
        prev_proj[1]()

        for p in reversed(pools):
            p.__exit__(None, None, None)

    nc.compile()
    return nc


def _host_masks():
    bf = ml_dtypes.bfloat16
    kt = np.arange(128)[:, None]
    qi = np.arange(128)[None, :]
    tri0 = (qi <= kt).astype(bf)          # r=0 keep
    tri2 = (kt <= qi).astype(bf)          # r=2 keep
    zeros = np.zeros((128, 128), bf)

    masks = []
    for c in range(NCORES):
        m = np.empty((128, 512), bf)
        m[:, 0:128] = tri2
        m[:, 128:256] = tri0
        m[:, 256:384] = zeros if c == 0 else tri0
        m[:, 384:512] = zeros if c == NCORES - 1 else tri2
        masks.append(m)
    return masks


def _host_inputs(query, key, value, Wq, Wk, Wv, Wo):
    bf = ml_dtypes.bfloat16
    q2 = np.ascontiguousarray(query.reshape(S, D))
    k2 = np.asarray(key).reshape(S, D)
    v2 = np.asarray(value).reshape(S, D)
    kpad = np.zeros((S + 2 * WIN, D), np.float32)
    kpad[WIN:WIN + S] = k2
    vpad = np.zeros((S + 2 * WIN, D), np.float32)
    vpad[WIN:WIN + S] = v2

    wq = np.ascontiguousarray(Wq.astype(bf))
    wk = np.ascontiguousarray(Wk.astype(bf))
    wv = np.ascontiguousarray(Wv.astype(bf))
    wo = np.ascontiguousarray(Wo.astype(bf))
    masks = _host_masks()

    in_maps = []
    for c in range(NCORES):
        s0 = c * SL
        qt = np.ascontiguousarray(q2[s0:s0 + SL].T.astype(bf))
        ktc = np.ascontiguousarray(kpad[s0:s0 + SK].T.astype(bf))
        vtc = np.ascontiguousarray(vpad[s0:s0 + SK].T.astype(bf))
        in_maps.append({
            "qt": qt, "kt": ktc, "vt": vtc,
            "wq": wq, "wk": wk, "wv": wv, "wo": wo,
            "msk": masks[c],
        })
    return in_maps


def kernel(query, key, value, Wq, Wk, Wv, Wo):
    global LAST_RESULT
    if "nc" not in _CACHE:
        _CACHE["nc"] = _build_nc()
    nc = _CACHE["nc"]
    in_maps = _host_inputs(
        np.asarray(query), np.asarray(key), np.asarray(value),
        np.asarray(Wq), np.asarray(Wk), np.asarray(Wv), np.asarray(Wo),
    )
    trace = os.environ.get("KERNEL_TRACE", "0") == "1"
    try:
        res = run_bass_kernel_spmd(
            nc, in_maps, core_ids=list(range(NCORES)), trace=trace
        )
    except ModuleNotFoundError:
        res = run_bass_kernel_spmd(
            nc, in_maps, core_ids=list(range(NCORES)), trace=False
        )
    LAST_RESULT = res
    out = np.concatenate([res.results[c]["out"] for c in range(NCORES)], axis=0)
    return out.reshape(1, S, D).astype(np.float32)
